# revision 1
# baseline (speedup 1.0000x reference)
"""Additive (Bahdanau) attention on 8 TRN2 NeuronCores.

scores[b,t,s] = softmax_s( sum_d v[d] * tanh(e1[b,s,d] + e2[b,t,d]) )  with mask
  e1 = enc @ We.T   [B,S,D]
  e2 = dec @ Wd.T   [B,T,D]

Sharding: pure data-parallel, core k handles batch b=k//2, t-half k%2
(128 t-rows each). No collectives.

v4: Fourier factorization. With y = (pi/L)*x, L=12 covering |e1+e2|:
  tanh(x) ~= y/pi + sum_k b_k sin(k y),   sin(k(y1+y2)) expands to
  sin(k y1)cos(k y2) + cos(k y1)sin(k y2) — each a rank-D matmul over d!
So the whole [T,S,D] elementwise tanh collapses into 2K matmuls of
[128t x 512d x kp_s] that accumulate scores DIRECTLY as [t(part), s(free)]
in one PSUM bank (no per-row gather, no partition-respread).

Basis generation per k (exact range reduction, Sin ACT only valid [-pi,pi]):
  y1p = y1+pi >= 0 (set up once):  t = (k*y1p) mod 2pi   [1 DVE op]
  S~1 = Sin(t - pi) = (-1)^(k-1) sin(k y1)               [ScalarE]
  y1pp = y1+2pi:  t = (k*y1pp + pi/2) mod 2pi            [2 DVE ops]
  C~1 = Sin(t - pi) = -cos(k y1)
  signs fold into host coeffs vb_k = -(-1)^(k-1) b_k v.
e2-side mods run on GpSimd (otherwise idle). Mask compression as v3 (host
gathers unmasked s-columns, pads killed by a -60000 K=1 matmul row).
Linear term via ones-operand matmuls. Softmax reads PSUM directly.
"""

import numpy as np

B, T, S, D = 4, 256, 512, 512
C = 2 * D
NCORES = 8
TLOC = 128  # t-rows per core
KF = 16  # Fourier terms
LDOM = 12.0  # tanh approx domain [-L, L]
BUFS1 = 3  # staging pool depth (arg tiles)
BUFS2 = 8  # basis pool depth
FOLD_GPSIMD = False  # vb folds on GpSimd vs DVE
TC1_GPSIMD = False  # cos-path pre-mod (mult+add) op on GpSimd

_CACHE = {}

_PI = float(np.pi)


def fourier_coeffs(K=KF, L=LDOM):
    key = ("coef", K, L)
    if key not in _CACHE:
        xg = np.linspace(-L, L, 6001)
        w = np.exp(-xg ** 2 / (2 * 1.66 ** 2)) + 1e-3
        resid = np.tanh(xg) - xg / L
        A = np.stack([np.sin(k * np.pi / L * xg) for k in range(1, K + 1)], 1)
        Wc = np.sqrt(w)[:, None]
        b, *_ = np.linalg.lstsq(A * Wc, resid * Wc[:, 0], rcond=None)
        _CACHE[key] = b
    return _CACHE[key]


def _build(kp):
    import concourse.mybir as mybir
    from concourse import bacc
    from concourse.tile import TileContext

    f32 = mybir.dt.float32
    f16 = mybir.dt.float16
    AF = mybir.ActivationFunctionType
    ALU = mybir.AluOpType

    nc = bacc.Bacc()
    encT_d = nc.declare_dram_parameter("encT", [C, kp], f16, isOutput=False)
    decT_d = nc.declare_dram_parameter("decT", [D, TLOC], f16, isOutput=False)
    WeTs_d = nc.declare_dram_parameter("WeTs", [C, D], f16, isOutput=False)
    WdTs_d = nc.declare_dram_parameter("WdTs", [D, D], f16, isOutput=False)
    vb_d = nc.declare_dram_parameter("vb", [128, D // 128, KF], f32,
                                     isOutput=False)
    vlin_d = nc.declare_dram_parameter("vlin", [128, D // 128], f32,
                                       isOutput=False)
    pad_d = nc.declare_dram_parameter("padrow", [1, kp], f16, isOutput=False)
    out_d = nc.declare_dram_parameter("out", [TLOC, kp], f32, isOutput=True)

    ND = D // 128  # 4 d-tiles
    NC_ = C // 128  # 8 c-tiles
    TWO_PI = 2 * _PI

    with TileContext(nc) as tc:
        with tc.tile_pool(name="persist", bufs=1) as pp:
            dma = nc.default_dma_engine

            WeT_sb = []
            encT_sb = []
            for ci in range(NC_):
                tw = pp.tile([128, D], f16, tag=f"WeT{ci}")
                dma.dma_start(out=tw, in_=WeTs_d[ci * 128:(ci + 1) * 128, :])
                WeT_sb.append(tw)
                te = pp.tile([128, kp], f16, tag=f"encT{ci}")
                dma.dma_start(out=te, in_=encT_d[ci * 128:(ci + 1) * 128, :])
                encT_sb.append(te)
            WdT_sb = []
            decT_sb = []
            for di in range(ND):
                tw = pp.tile([128, D], f16, tag=f"WdT{di}")
                dma.dma_start(out=tw, in_=WdTs_d[di * 128:(di + 1) * 128, :])
                WdT_sb.append(tw)
                td = pp.tile([128, TLOC], f16, tag=f"decT{di}")
                dma.dma_start(out=td, in_=decT_d[di * 128:(di + 1) * 128, :])
                decT_sb.append(td)
            vb_sb = pp.tile([128, ND, KF], f32, tag="vb")
            dma.dma_start(out=vb_sb, in_=vb_d[:, :, :])
            vlin_sb = pp.tile([128, ND], f32, tag="vlin")
            dma.dma_start(out=vlin_sb, in_=vlin_d[:, :])
            pad_sb = pp.tile([1, kp], f16, tag="padrow")
            dma.dma_start(out=pad_sb, in_=pad_d[:, :])

            ones_t = pp.tile([128, TLOC], f16, tag="ones_t")
            nc.vector.memset(ones_t, 1.0)
            ones_s = pp.tile([128, kp], f16, tag="ones_s")
            nc.vector.memset(ones_s, 1.0)
            ones_c = pp.tile([1, TLOC], f16, tag="ones_c")
            nc.vector.memset(ones_c, 1.0)
            negpi = pp.tile([128, 1], f32, tag="negpi")
            nc.vector.memset(negpi, -_PI)

            # y1 = (pi/L)*e1 in [d, s'] layout; offsets +pi / +2pi for the
            # sin/cos range-reduction paths; vlin/vy folds for linear term
            y1_sb = pp.tile([128, ND, kp], f32, tag="y1_sb")
            vy1 = pp.tile([128, ND, kp], f16, tag="vy1")
            y2_sb = pp.tile([128, ND, TLOC], f32, tag="y2_sb")
            vy2 = pp.tile([128, ND, TLOC], f16, tag="vy2")

            with tc.tile_pool(name="mm_psum", bufs=2, space="PSUM") as mmp:
                for dj in range(ND):
                    ps = mmp.tile([128, kp], f32, tag="pe1")
                    for ci in range(NC_):
                        nc.tensor.matmul(
                            ps,
                            WeT_sb[ci][:, dj * 128:(dj + 1) * 128],
                            encT_sb[ci],
                            start=(ci == 0),
                            stop=(ci == NC_ - 1),
                        )
                    nc.vector.tensor_copy(y1_sb[:, dj, :], ps)
                    nc.vector.tensor_scalar_mul(
                        vy1[:, dj, :], ps, vlin_sb[:, dj:dj + 1])
                for ej in range(ND):
                    ps = mmp.tile([128, TLOC], f32, tag="pe2")
                    for di in range(ND):
                        nc.tensor.matmul(
                            ps,
                            WdT_sb[di][:, ej * 128:(ej + 1) * 128],
                            decT_sb[di],
                            start=(di == 0),
                            stop=(di == ND - 1),
                        )
                    nc.vector.tensor_copy(y2_sb[:, ej, :], ps)
                    nc.vector.tensor_scalar_mul(
                        vy2[:, ej, :], ps, vlin_sb[:, ej:ej + 1])

            with (
                tc.tile_pool(name="stg1", bufs=BUFS1) as sg1,
                tc.tile_pool(name="stg2", bufs=BUFS1) as sg2,
                tc.tile_pool(name="bas", bufs=BUFS2) as bp,
                tc.tile_pool(name="sc_psum", bufs=1, space="PSUM") as scp,
            ):
                sc = scp.tile([TLOC, kp], f32, tag="sc")

                # linear term: sum_d vlin_d*(y1[s,d]+y2[t,d])
                first = True
                for dj in range(ND):
                    nc.tensor.matmul(sc, ones_t, vy1[:, dj, :],
                                     start=first, stop=False)
                    first = False
                    nc.tensor.matmul(sc, vy2[:, dj, :], ones_s,
                                     start=False, stop=False)

                ALUm, ALUs = ALU.mult, ALU.subtract

                def seed_side(y_sb, n_free, eng_tt):
                    # SC_1 = [sin(y)|cos(y)]; cos via 1-2*sin^2(y/2);
                    # SC_2 = [2 s1 c1 | 1-2 s1^2]; stride-2 Chebyshev uses
                    # tcd = [2cos(2y)|2cos(2y)] so odd/even k form two
                    # INDEPENDENT recurrence chains (better pipelining, less
                    # fp16 compounding).
                    sc1 = bp.tile([128, 2, ND, n_free], f16, tag=f"sc{n_free}")
                    nc.scalar.activation(out=sc1[:, 0], in_=y_sb, func=AF.Sin)
                    u = bp.tile([128, ND, n_free], f16, tag=f"u{n_free}")
                    nc.scalar.activation(out=u, in_=y_sb, func=AF.Sin,
                                         scale=0.5)
                    u2 = bp.tile([128, ND, n_free], f16, tag=f"u2{n_free}")
                    nc.vector.tensor_tensor(u2, u, u, op=ALUm)
                    nc.vector.tensor_scalar(
                        out=sc1[:, 1], in0=u2, scalar1=-2.0, scalar2=1.0,
                        op0=ALUm, op1=ALU.add)
                    sc2 = bp.tile([128, 2, ND, n_free], f16, tag=f"sc{n_free}")
                    s1sq = bp.tile([128, ND, n_free], f16, tag=f"u2{n_free}")
                    nc.vector.tensor_tensor(s1sq, sc1[:, 0], sc1[:, 0],
                                            op=ALUm)
                    nc.vector.tensor_scalar(
                        out=sc2[:, 1], in0=s1sq, scalar1=-2.0, scalar2=1.0,
                        op0=ALUm, op1=ALU.add)
                    tmp2 = bp.tile([128, ND, n_free], f16, tag=f"u{n_free}")
                    nc.vector.tensor_scalar_mul(tmp2, sc1[:, 1], 2.0)
                    nc.vector.tensor_tensor(sc2[:, 0], tmp2, sc1[:, 0],
                                            op=ALUm)
                    tcd = pp.tile([128, 2, ND, n_free], f16, tag=f"tcd{n_free}")
                    nc.vector.tensor_scalar_mul(tcd[:, 0], sc2[:, 1], 2.0)
                    nc.vector.tensor_copy(tcd[:, 1], tcd[:, 0])
                    return sc1, sc2, tcd

                sc1_1, sc1_2, tcd1 = seed_side(y1_sb, kp, nc.vector)
                sc2_1, sc2_2, tcd2 = seed_side(y2_sb, TLOC, nc.gpsimd)

                def minus_one_seeds(sc1t, n_free):
                    # odd chain k-2 seed: [sin(-y)|cos(-y)] = [-s1|c1]
                    # even chain k-2 seed: [sin(0)|cos(0)] = [0|1]
                    scm1 = bp.tile([128, 2, ND, n_free], f16,
                                   tag=f"sc{n_free}")
                    nc.vector.tensor_scalar_mul(scm1[:, 0], sc1t[:, 0], -1.0)
                    nc.vector.tensor_copy(scm1[:, 1], sc1t[:, 1])
                    sc0 = bp.tile([128, 2, ND, n_free], f16,
                                  tag=f"sc{n_free}")
                    nc.vector.memset(sc0[:, 0], 0.0)
                    nc.vector.memset(sc0[:, 1], 1.0)
                    return scm1, sc0

                scm1_1, sc0_1 = minus_one_seeds(sc1_1, kp)
                scm1_2, sc0_2 = minus_one_seeds(sc2_1, TLOC)

                # chains[par] = [k-2 tile, k tile]
                ch1 = {1: [scm1_1, sc1_1], 0: [sc0_1, sc1_2]}
                ch2 = {1: [scm1_2, sc2_1], 0: [sc0_2, sc2_2]}
                for k in range(1, KF + 1):
                    par = k % 2
                    if k <= 2:
                        SC1 = ch1[par][1]
                        SC2 = ch2[par][1]
                    else:
                        SC1 = bp.tile([128, 2, ND, kp], f16, tag=f"sc{kp}")
                        nc.vector.tensor_tensor(SC1, tcd1, ch1[par][1],
                                                op=ALUm)
                        nc.vector.tensor_tensor(SC1, SC1, ch1[par][0],
                                                op=ALUs)
                        SC2 = bp.tile([128, 2, ND, TLOC], f16,
                                      tag=f"sc{TLOC}")
                        nc.gpsimd.tensor_tensor(SC2, tcd2, ch2[par][1],
                                                op=ALUm)
                        nc.gpsimd.tensor_tensor(SC2, SC2, ch2[par][0],
                                                op=ALUs)
                        ch1[par] = [ch1[par][1], SC1]
                        ch2[par] = [ch2[par][1], SC2]
                    vSC1 = bp.tile([128, 2, ND, kp], f16, tag="vSC1")
                    for dj in range(ND):
                        # sin-side folds on the (otherwise idle) ScalarE,
                        # cos-side on DVE
                        nc.scalar.activation(
                            out=vSC1[:, 0, dj, :], in_=SC1[:, 0, dj, :],
                            func=AF.Copy, scale=vb_sb[:, dj, k - 1:k])
                        nc.vector.tensor_scalar_mul(
                            vSC1[:, 1, dj, :], SC1[:, 1, dj, :],
                            vb_sb[:, dj, k - 1:k])
                    for dj in range(ND):
                        # sin1*cos2: lhsT = C2, rhs = vb*S1
                        nc.tensor.matmul(sc, SC2[:, 1, dj, :],
                                         vSC1[:, 0, dj, :],
                                         start=False, stop=False)
                        # cos1*sin2: lhsT = S2, rhs = vb*C1
                        nc.tensor.matmul(sc, SC2[:, 0, dj, :],
                                         vSC1[:, 1, dj, :],
                                         start=False, stop=False)

                # kill pad columns before softmax: rank-1 row of -60000
                nc.tensor.matmul(sc, ones_c, pad_sb, start=False, stop=True)

                with tc.tile_pool(name="smx", bufs=1) as wp:
                    negmax = wp.tile([TLOC, 1], f32, tag="negmax")
                    nc.vector.reduce_max(
                        negmax, sc, axis=mybir.AxisListType.X, negate=True)
                    expt = wp.tile([TLOC, kp], f32, tag="expt")
                    sums = wp.tile([TLOC, 1], f32, tag="sums")
                    nc.scalar.activation(
                        out=expt, in_=sc, func=AF.Exp,
                        bias=negmax, scale=1.0, accum_out=sums)
                    rec = wp.tile([TLOC, 1], f32, tag="rec")
                    nc.vector.reciprocal(rec, sums)
                    outt = wp.tile([TLOC, kp], f32, tag="outt")
                    nc.vector.tensor_scalar_mul(outt, expt, rec)
                    dma.dma_start(out=out_d[:, :], in_=outt)

    return nc


def _get_nc(kp):
    key = ("nc", kp)
    if key not in _CACHE:
        nc = _build(kp)
        nc.finalize()  # Bacc legalization (wait splitting etc.) + freeze
        _CACHE[key] = nc
    return _CACHE[key]


def make_in_maps(decoder_outputs, encoder_outputs, mask, We, Wd, v):
    f32 = np.float32
    f16 = np.float16
    mask = np.asarray(mask)
    keep_idx = [np.where(~mask[b])[0] for b in range(B)]
    nkeep = [len(ix) for ix in keep_idx]
    kp = max(16, -16 * (-max(nkeep) // 16))  # round up to multiple of 16

    om = _PI / LDOM
    bcoef = fourier_coeffs()
    # signs: S~=(-1)^(k-1) sin, C~=-cos on both sides; term needs
    # b_k(sin1 cos2 + cos1 sin2) = -sigma_k b_k (S~1 C~2 + C~1 S~2)
    vb = np.empty((128, D // 128, KF), f32)
    for k in range(1, KF + 1):
        vb[:, :, k - 1] = bcoef[k - 1] * \
            v.astype(f32).reshape(D // 128, 128).T
    vlin = np.ascontiguousarray(
        (v.astype(f32) / _PI).reshape(D // 128, 128).T)

    WeTs = np.ascontiguousarray((om * We).T.astype(f16))
    WdTs = np.ascontiguousarray((om * Wd).T.astype(f16))

    in_maps = []
    for k in range(NCORES):
        b, th = k // 2, k % 2
        ix = keep_idx[b]
        ix_pad = np.concatenate(
            [ix, np.full(kp - len(ix), ix[-1], dtype=ix.dtype)]
        )
        encT_kept = np.ascontiguousarray(
            encoder_outputs[b].astype(f16).T[:, ix_pad]
        )
        pad = np.concatenate(
            [np.zeros(len(ix), f16), np.full(kp - len(ix), f16(-60000))]
        )
        in_maps.append({
            "encT": encT_kept,
            "decT": np.ascontiguousarray(
                decoder_outputs[b, th * TLOC:(th + 1) * TLOC].astype(f16).T
            ),
            "WeTs": WeTs,
            "WdTs": WdTs,
            "vb": vb,
            "vlin": vlin,
            "padrow": pad.reshape(1, kp),
        })
    meta = {"kp": kp, "keep_idx": keep_idx, "nkeep": nkeep}
    return in_maps, meta


def assemble(results, meta):
    full = np.zeros((B, T, S), dtype=np.float32)
    for k in range(NCORES):
        b, th = k // 2, k % 2
        ix = meta["keep_idx"][b]
        out = results[k]["out"]  # [TLOC, kp]
        full[b, th * TLOC:(th + 1) * TLOC, ix] = out[:, :len(ix)].T
    return full


def kernel(decoder_outputs, encoder_outputs, mask, We, Wd, v):
    from concourse.bass_utils import run_bass_kernel_spmd

    in_maps, meta = make_in_maps(
        decoder_outputs, encoder_outputs, mask, We, Wd, v
    )
    nc = _get_nc(meta["kp"])
    res = run_bass_kernel_spmd(nc, in_maps, core_ids=list(range(NCORES)))
    return assemble(res.results, meta)



# revision 7
# speedup vs baseline: 1.1325x; 1.1325x over previous
"""Additive (Bahdanau) attention on 8 TRN2 NeuronCores.

scores[b,t,s] = softmax_s( sum_d v[d] * tanh(e1[b,s,d] + e2[b,t,d]) )  with mask
  e1 = enc @ We.T   [B,S,D]
  e2 = dec @ Wd.T   [B,T,D]

Sharding: pure data-parallel, core k handles batch b=k//2, t-half k%2
(128 t-rows each). No collectives.

v5: Fourier factorization, recurrence basis. With y = (pi/L)*x:
  tanh(x) ~= x/L + sum_k b_k sin(k(y1+y2));  sin(k(y1+y2)) expands into
  sin(ky1)cos(ky2)+cos(ky1)sin(ky2) — rank-D matmuls accumulating scores
  directly as [t(part), s(free)] in one PSUM bank.
Basis sin/cos(k y) for the s-side and t-side PACKED in one tile
[128, 2(trig), ND, W=kp+TLOC] f16, built WITHOUT range reduction (no mod
ALU on this ISA):
  k=1: ACT Sin reads e-PSUM directly (y in [-pi,pi]); cos via 1-2sin^2(y/2)
  even k=2j: double angle, 4 DVE f16 2x ops
  odd  k:    Chebyshev step SC_k = 2cos(y)*SC_{k-1} - SC_{k-2}, 2 DVE ops
Folds (w_k = b_k*v per d) hit only the t-part, split DVE/Pool. Linear term
via host-precomputed wle=We'^T vlin, wld=Wd'^T vlin -> rank-1 rows merged
with the mask pad-kill into ONE rank-2 f32 matmul. Softmax skips the max
subtraction (scores bounded), exp+accum on ACT, normalize on DVE.
Mask compression as v4 (host gathers unmasked s-columns; pad columns get
-60000 via the pad row).
"""

import numpy as np

B, T, S, D = 4, 256, 512, 512
C = 2 * D
NCORES = 8
TLOC = 128  # t-rows per core
KF = 10  # Fourier terms
LDOM = 10.0  # tanh approx domain [-L, L]
ND = D // 128  # 4 d-tiles
NC_ = C // 128  # 8 c-tiles
POOL_FOLD_KS = frozenset({1, 2, 3, 4, 5, 6, 7})  # fold ks on GpSimd
BAS_BUFS = 10
TMP_BUFS = 3
VT_BUFS = 4

_CACHE = {}

_PI = float(np.pi)


def fourier_coeffs(K=KF, L=LDOM):
    key = ("coef", K, L)
    if key not in _CACHE:
        xg = np.linspace(-L, L, 6001)
        w = np.exp(-xg ** 2 / (2 * 1.66 ** 2)) + 1e-3
        resid = np.tanh(xg) - xg / L
        A = np.stack([np.sin(k * np.pi / L * xg) for k in range(1, K + 1)], 1)
        Wc = np.sqrt(w)[:, None]
        b, *_ = np.linalg.lstsq(A * Wc, resid * Wc[:, 0], rcond=None)
        _CACHE[key] = b
    return _CACHE[key]


def _build(kp, repeat=1):
    import concourse.mybir as mybir
    from concourse import bacc
    from concourse.tile import TileContext

    f32 = mybir.dt.float32
    f16 = mybir.dt.float16
    AF = mybir.ActivationFunctionType
    ALU = mybir.AluOpType

    W = kp + TLOC

    nc = bacc.Bacc()
    encT_d = nc.declare_dram_parameter("encT", [128, NC_, kp], f16,
                                       isOutput=False)
    decT_d = nc.declare_dram_parameter("decT", [128, ND, TLOC], f16,
                                       isOutput=False)
    WeTs_d = nc.declare_dram_parameter("WeTs", [128, NC_, D], f16,
                                       isOutput=False)
    WdTs_d = nc.declare_dram_parameter("WdTs", [128, ND, D], f16,
                                       isOutput=False)
    wf_d = nc.declare_dram_parameter("wf", [128, ND, KF], f32,
                                     isOutput=False)
    wle_d = nc.declare_dram_parameter("wle", [128, NC_], f16, isOutput=False)
    wld_d = nc.declare_dram_parameter("wld", [128, ND], f16, isOutput=False)
    pad_d = nc.declare_dram_parameter("padrow", [1, kp], f32, isOutput=False)
    out_d = nc.declare_dram_parameter("out", [TLOC, kp], f32, isOutput=True)

    with TileContext(nc) as tc:
        with tc.tile_pool(name="persist", bufs=1) as pp:
            dma = nc.default_dma_engine

            WdT_sb = pp.tile([128, ND, D], f16, tag="WdT")
            dma.dma_start(out=WdT_sb, in_=WdTs_d[:, :, :])
            decT_sb = pp.tile([128, ND, TLOC], f16, tag="decT")
            dma.dma_start(out=decT_sb, in_=decT_d[:, :, :])
            WeT_sb = pp.tile([128, NC_, D], f16, tag="WeT")
            dma.dma_start(out=WeT_sb, in_=WeTs_d[:, :, :])
            encT_sb = pp.tile([128, NC_, kp], f16, tag="encT")
            dma.dma_start(out=encT_sb, in_=encT_d[:, :, :])
            wf_sb = pp.tile([128, ND, KF], f32, tag="wf")
            dma.dma_start(out=wf_sb, in_=wf_d[:, :, :])
            wle_sb = pp.tile([128, NC_], f16, tag="wle")
            dma.dma_start(out=wle_sb, in_=wle_d[:, :])
            wld_sb = pp.tile([128, ND], f16, tag="wld")
            dma.dma_start(out=wld_sb, in_=wld_d[:, :])
            pad_sb = pp.tile([1, kp], f32, tag="padrow")
            dma.dma_start(out=pad_sb, in_=pad_d[:, :])

            for _rep in range(repeat):
                with (
                    tc.tile_pool(name="pro_psum", bufs=1, space="PSUM") as qp,
                    tc.tile_pool(name="seed", bufs=1) as sp,
                ):
                    # ---- prologue: e2, e1, r2, r1 matmuls; seed Sins ----
                    sc1 = sp.tile([128, 2, ND, W], f16, tag="sc1")
                    u = sp.tile([128, ND, W], f16, tag="useed")
                    ones_s = sp.tile([1, kp], f32, tag="ones_s")
                    ones_c = sp.tile([1, TLOC], f32, tag="ones_c")
                    r1pad = sp.tile([1, kp], f32, tag="r1pad")
                    r2row = sp.tile([1, TLOC], f32, tag="r2row")

                    pe2 = qp.tile([128, ND, TLOC], f32, tag="pe2")
                    for ej in range(ND):
                        for di in range(ND):
                            nc.tensor.matmul(
                                pe2[:, ej, :],
                                WdT_sb[:, di, ej * 128:(ej + 1) * 128],
                                decT_sb[:, di, :],
                                start=(di == 0), stop=(di == ND - 1),
                            )
                        nc.scalar.activation(
                            out=sc1[:, 0, ej, kp:], in_=pe2[:, ej, :],
                            func=AF.Sin)
                        nc.scalar.activation(
                            out=u[:, ej, kp:], in_=pe2[:, ej, :],
                            func=AF.Sin, scale=0.5)
                    pr2 = qp.tile([1, TLOC], f32, tag="pr2")
                    for di in range(ND):
                        nc.tensor.matmul(
                            pr2, wld_sb[:, di:di + 1], decT_sb[:, di, :],
                            start=(di == 0), stop=(di == ND - 1))
                    pe1 = []
                    for dj in range(ND):
                        ps = qp.tile([128, kp], f32, tag=f"pe1_{dj}")
                        pe1.append(ps)
                        for ci in range(NC_):
                            nc.tensor.matmul(
                                ps,
                                WeT_sb[:, ci, dj * 128:(dj + 1) * 128],
                                encT_sb[:, ci, :],
                                start=(ci == 0), stop=(ci == NC_ - 1),
                            )
                        nc.scalar.activation(
                            out=sc1[:, 0, dj, :kp], in_=ps, func=AF.Sin)
                        nc.scalar.activation(
                            out=u[:, dj, :kp], in_=ps, func=AF.Sin, scale=0.5)
                    pr1 = qp.tile([1, kp], f32, tag="pr1")
                    for ci in range(NC_):
                        nc.tensor.matmul(
                            pr1, wle_sb[:, ci:ci + 1], encT_sb[:, ci, :],
                            start=(ci == 0), stop=(ci == NC_ - 1))

                    # linear-term + pad-kill rank-1 operands
                    nc.vector.memset(ones_s, 1.0)
                    nc.vector.memset(ones_c, 1.0)
                    nc.vector.tensor_tensor(r1pad, pr1, pad_sb, op=ALU.add)
                    nc.vector.tensor_copy(r2row, pr2)

                    # seeds: C1 = 1-2*u^2, tcd = 2*C1 (broadcastable [.,1,.])
                    q0 = sp.tile([128, ND, W], f16, tag="q0")
                    nc.vector.tensor_tensor(q0, u, u, op=ALU.mult)
                    nc.vector.tensor_scalar(
                        out=sc1[:, 1], in0=q0, scalar1=-2.0, scalar2=1.0,
                        op0=ALU.mult, op1=ALU.add)
                    tcd = sp.tile([128, 1, ND, W], f16, tag="tcd")
                    nc.vector.tensor_scalar_mul(tcd[:, 0], sc1[:, 1], 2.0)

                    with (
                        tc.tile_pool(name="sc_psum", bufs=1,
                                     space="PSUM") as scp,
                        tc.tile_pool(name="bas", bufs=BAS_BUFS) as bp,
                        tc.tile_pool(name="tmp", bufs=TMP_BUFS) as tp,
                        tc.tile_pool(name="vt", bufs=VT_BUFS) as vp,
                    ):
                        sc = scp.tile([TLOC, kp], f32, tag="sc")
                        nc.tensor.matmul(sc, ones_c, r1pad,
                                         start=True, stop=False)
                        nc.tensor.matmul(sc, r2row, ones_s,
                                         start=False, stop=False)

                        SC = {1: sc1}
                        tcd_b = tcd.broadcast_to([128, 2, ND, W])

                        def folds_and_mm(k, last=False):
                            SCk = SC[k]
                            eng = (nc.gpsimd if k in POOL_FOLD_KS
                                   else nc.vector)
                            vt = vp.tile([128, 2, ND, TLOC], f16,
                                         tag="vt")
                            for dj in range(ND):
                                eng.tensor_scalar_mul(
                                    vt[:, :, dj, :], SCk[:, :, dj, kp:],
                                    wf_sb[:, dj, k - 1:k])
                            for dj in range(ND):
                                nc.tensor.matmul(
                                    sc, vt[:, 1, dj, :],
                                    SCk[:, 0, dj, :kp],
                                    start=False, stop=False)
                                nc.tensor.matmul(
                                    sc, vt[:, 0, dj, :],
                                    SCk[:, 1, dj, :kp],
                                    start=False,
                                    stop=(last and dj == ND - 1))

                        folds_and_mm(1)
                        for k in range(2, KF + 1):
                            SCk = bp.tile([128, 2, ND, W], f16, tag="SC")
                            if k % 2 == 0:
                                j = k // 2
                                ts_ = tp.tile([128, ND, W], f16, tag="ts")
                                nc.vector.tensor_tensor(
                                    ts_, SC[j][:, 0], SC[j][:, 1],
                                    op=ALU.mult)
                                nc.vector.tensor_scalar_mul(
                                    SCk[:, 0], ts_, 2.0)
                                qq = tp.tile([128, ND, W], f16, tag="qq")
                                nc.vector.tensor_tensor(
                                    qq, SC[j][:, 0], SC[j][:, 0],
                                    op=ALU.mult)
                                nc.vector.tensor_scalar(
                                    out=SCk[:, 1], in0=qq, scalar1=-2.0,
                                    scalar2=1.0, op0=ALU.mult, op1=ALU.add)
                            else:
                                tmp = tp.tile([128, 2, ND, W], f16,
                                              tag="tmp2")
                                nc.vector.tensor_tensor(
                                    tmp, SC[k - 1], tcd_b, op=ALU.mult)
                                nc.vector.tensor_tensor(
                                    SCk, tmp, SC[k - 2], op=ALU.subtract)
                            SC[k] = SCk
                            folds_and_mm(k, last=(k == KF))

                        # ---- softmax (no max subtraction; pads -> 0) ----
                        with tc.tile_pool(name="smx", bufs=1) as wp:
                            expt = wp.tile([TLOC, kp], f32, tag="expt")
                            sums = wp.tile([TLOC, 1], f32, tag="sums")
                            nc.scalar.activation(
                                out=expt, in_=sc, func=AF.Exp,
                                accum_out=sums)
                            rec = wp.tile([TLOC, 1], f32, tag="rec")
                            nc.vector.reciprocal(rec, sums)
                            outt = wp.tile([TLOC, kp], f32, tag="outt")
                            nc.vector.tensor_scalar_mul(outt, expt, rec)
                            dma.dma_start(out=out_d[:, :], in_=outt)

    return nc


def _get_nc(kp, repeat=1):
    key = ("nc", kp, repeat)
    if key not in _CACHE:
        nc = _build(kp, repeat=repeat)
        nc.finalize()
        _CACHE[key] = nc
    return _CACHE[key]


def _pm(x, n):
    """[n*128, m] -> partition-major [128, n, m]."""
    m = x.shape[1] if x.ndim > 1 else 1
    return np.ascontiguousarray(
        x.reshape(n, 128, -1).transpose(1, 0, 2).reshape(128, n, m)
    )


def make_in_maps(decoder_outputs, encoder_outputs, mask, We, Wd, v):
    f32 = np.float32
    f16 = np.float16
    mask = np.asarray(mask)
    keep_idx = [np.where(~mask[b])[0] for b in range(B)]
    nkeep = [len(ix) for ix in keep_idx]
    kp = max(16, -16 * (-max(nkeep) // 16))  # round up to multiple of 16

    om = _PI / LDOM
    bcoef = fourier_coeffs()
    vf = np.asarray(v).astype(np.float64)

    wf = np.empty((128, ND, KF), f32)
    vpm = vf.reshape(ND, 128).T  # [128, ND]
    for k in range(1, KF + 1):
        wf[:, :, k - 1] = (bcoef[k - 1] * vpm).astype(f32)

    WeS = (om * np.asarray(We).astype(np.float64))  # [D, C]
    WdS = (om * np.asarray(Wd).astype(np.float64))  # [D, D]
    wle = _pm((WeS.T @ (vf / _PI)).astype(f16).reshape(C, 1), NC_)[:, :, 0]
    wld = _pm((WdS.T @ (vf / _PI)).astype(f16).reshape(D, 1), ND)[:, :, 0]
    WeTs = _pm(np.ascontiguousarray(WeS.T).astype(f16), NC_)  # [128,NC_,D]
    WdTs = _pm(np.ascontiguousarray(WdS.T).astype(f16), ND)  # [128,ND,D]

    in_maps = []
    for kcore in range(NCORES):
        b, th = kcore // 2, kcore % 2
        ix = keep_idx[b]
        ix_pad = np.concatenate(
            [ix, np.full(kp - len(ix), ix[-1], dtype=ix.dtype)]
        )
        encT_kept = np.ascontiguousarray(
            np.asarray(encoder_outputs)[b].astype(f16).T[:, ix_pad]
        )
        decT = np.ascontiguousarray(
            np.asarray(decoder_outputs)[b, th * TLOC:(th + 1) * TLOC]
            .astype(f16).T
        )
        pad = np.concatenate(
            [np.zeros(len(ix), f32), np.full(kp - len(ix), f32(-60000.0))]
        )
        in_maps.append({
            "encT": _pm(encT_kept, NC_),
            "decT": _pm(decT, ND),
            "WeTs": WeTs,
            "WdTs": WdTs,
            "wf": wf,
            "wle": wle,
            "wld": wld,
            "padrow": pad.reshape(1, kp),
        })
    meta = {"kp": kp, "keep_idx": keep_idx, "nkeep": nkeep}
    return in_maps, meta


def assemble(results, meta):
    full = np.zeros((B, T, S), dtype=np.float32)
    for kcore in range(NCORES):
        b, th = kcore // 2, kcore % 2
        ix = meta["keep_idx"][b]
        out = results[kcore]["out"]  # [TLOC, kp]
        full[b, th * TLOC:(th + 1) * TLOC, ix] = out[:, :len(ix)].T
    return full


def kernel(decoder_outputs, encoder_outputs, mask, We, Wd, v):
    from concourse.bass_utils import run_bass_kernel_spmd

    in_maps, meta = make_in_maps(
        decoder_outputs, encoder_outputs, mask, We, Wd, v
    )
    nc = _get_nc(meta["kp"])
    res = run_bass_kernel_spmd(nc, in_maps, core_ids=list(range(NCORES)))
    return assemble(res.results, meta)
